# revision 23
# baseline (speedup 1.0000x reference)
"""Trainium2 Bass kernel for EngramCodebook (vq_codebook).

reference semantics:
    pooled    = hidden_state.mean(axis=0)                 # [2048]
    d[s]      = || seed_bank[s] - pooled ||               # [4096]
    idx       = argmin(d)
    usage_new = usage_frequency + onehot(idx)
    recon     = broadcast(seed_bank[idx], (16384, 2048))

Distribution (8 NeuronCores, column-sharded):
    core i owns columns c_i = [256*i, 256*(i+1)) of hidden_state/seed_bank.
    - local: pool_chunk = mean over rows of hidden[:, c_i]
      (dual-queue loads cast to fp16, DVE running sum, PE ones-matmul)
    - local: score[s] = 2*sb[s,c_i].pool_chunk - ||sb[s,c_i]||^2
      computed entirely from a transposed fp16 seed bank: ACT squares +
      PE M=128 partition-sum matmuls for the norms, PE matmuls against the
      transposed pooled vector for the dots (sign flipped -> arg-MAX).
      fp16 keeps the score error ~0.1 vs a winner gap of ~18.
    - one 16 KB AllReduce combines partial scores across the 8 shards
    - local: max -> index -> dynamic-slice DMA fetches the winning seed row
      (exact f32 from DRAM) -> broadcast to recon[:, c_i]; usage/idx are
      computed identically on every core.

Score-tile layout: element (p, b) of the [128, 32] tiles is seed
    s = 128*b + p  (sbt columns are host-permuted so PE M-blocks line up).
"""

import os
import sys

sys.path.insert(0, "/opt/trn_rl_repo")

import numpy as np

N_CORES = 8
N_ROWS = 16384          # hidden_state rows
D = 2048                # state dim
S = 4096                # num seeds
C = D // N_CORES        # columns per core = 256

_CACHE = {}


def _build_program():
    from concourse import bacc, bass, mybir, tile

    f32 = mybir.dt.float32
    f16 = mybir.dt.float16
    i32 = mybir.dt.int32

    nc = bacc.Bacc("TRN2", target_bir_lowering=False, debug=False,
                   num_devices=N_CORES)

    h = nc.dram_tensor("h", [N_ROWS, C], f32, kind="ExternalInput")
    sbk = nc.dram_tensor("sbk", [S, C], f32, kind="ExternalInput")
    sbt = nc.dram_tensor("sbt", [C, S], f16, kind="ExternalInput")
    uf = nc.dram_tensor("uf", [S], f32, kind="ExternalInput")

    recon = nc.dram_tensor("recon", [N_ROWS, C], f32, kind="ExternalOutput")
    usage = nc.dram_tensor("usage", [S], f32, kind="ExternalOutput")
    sidx = nc.dram_tensor("sidx", [1, 1], f32, kind="ExternalOutput")

    # DRAM views; every DMA has >=1 KiB contiguous runs per partition.
    hv = h.ap().rearrange("(n p e) c -> n p (e c)", n=16, p=128, e=8)
    stv = sbt.ap().rearrange("(k p) s -> k p s", k=2, p=128)
    ufv = uf.ap().rearrange("(q p) -> q p", q=32, p=128)
    usv = usage.ap().rearrange("(q p) -> q p", q=32, p=128)
    rv = recon.ap().rearrange("(n p e) c -> n p (e c)", n=16, p=128, e=8)

    with tile.TileContext(nc) as tc:
        with (
            tc.tile_pool(name="hpool", bufs=4) as hpool,
            tc.tile_pool(name="accp", bufs=2) as accp,
            tc.tile_pool(name="persist", bufs=1) as persist,
            tc.tile_pool(name="small", bufs=1) as small,
            tc.tile_pool(name="psum_dot", bufs=1, space="PSUM") as psum_dot,
            tc.tile_pool(name="psum_sq", bufs=1, space="PSUM") as psum_sq,
            tc.tile_pool(name="psum_sm", bufs=3, space="PSUM") as psum_sm,
            tc.tile_pool(name="dram", bufs=1, space="DRAM") as dram,
        ):
            # ---- kick off the big loads first ----
            # transposed fp16 seed bank (sync queue), persists for the dots
            sbt_sb = persist.tile([128, 2 * S], f16)
            for k in range(2):
                nc.sync.dma_start(sbt_sb[:, k * S:(k + 1) * S], stv[k])

            uf_sb = persist.tile([32, 128], f32)
            nc.sync.dma_start(uf_sb[:], ufv[:])

            # hidden tiles: even tiles on sync (f32 + DVE cast), odd tiles on
            # gpsimd (cast-to-fp16 during DMA) so both DMA queues stay busy
            h_bf = []
            for n in range(16):
                if n % 2 == 0:
                    h_f = hpool.tile([128, 2048], f32, name="h_f")
                    nc.sync.dma_start(h_f[:], hv[n])
                    h_b = hpool.tile([128, 2048], f16, name="h_b")
                    nc.vector.tensor_copy(h_b[:], h_f[:])
                else:
                    h_b = hpool.tile([128, 2048], f16, name="h_b")
                    nc.gpsimd.dma_start(h_b[:], hv[n])
                h_bf.append(h_b)

            # ---- constants (scheduled into load-phase idle time) ----
            ones_kh = persist.tile([128, 1], f16)
            nc.vector.memset(ones_kh[:], 1.0)
            ones_k = persist.tile([128, 1], f32)
            nc.vector.memset(ones_k[:], 1.0)
            ones_m = persist.tile([1, 128], f32)
            nc.vector.memset(ones_m[:], 1.0)
            scale_11 = persist.tile([1, 1], f32)
            nc.vector.memset(scale_11[:], 1.0 / float(N_ROWS))

            iota_pb_i = small.tile([128, 32], i32)
            nc.gpsimd.iota(iota_pb_i[:], pattern=[[128, 32]], base=0,
                           channel_multiplier=1)
            iota_pb = persist.tile([128, 32], f32)
            nc.vector.tensor_copy(iota_pb[:], iota_pb_i[:])

            iota_nat_i = small.tile([32, 128], i32)
            nc.gpsimd.iota(iota_nat_i[:], pattern=[[1, 128]], base=0,
                           channel_multiplier=128)
            iota_nat = persist.tile([32, 128], f32)
            nc.vector.tensor_copy(iota_nat[:], iota_nat_i[:])

            # identity matrix for PE transposes of [128,1] vectors
            iota_id_i = small.tile([128, 128], i32)
            nc.gpsimd.iota(iota_id_i[:], pattern=[[1, 128]], base=0,
                           channel_multiplier=-1)
            idf = small.tile([128, 128], f32)
            nc.vector.tensor_copy(idf[:], iota_id_i[:])
            ident = persist.tile([128, 128], f32)
            nc.vector.tensor_scalar(ident[:], idf[:], 0.0, None,
                                    op0=mybir.AluOpType.is_equal)

            # ---- seed norms from the fp16 transposed bank (load-phase) ----
            # sq[p, b] = sum_c sbt[c, 128b+p]^2 : ACT squares, PE M=128
            # partition-sum matmuls. Runs while the hidden tiles stream in.
            sq_psum = psum_sq.tile([128, 32], f32)
            sqd = []
            for k in range(2):
                sqd_k = persist.tile([128, S], f16, name=f"sqd{k}")
                nc.scalar.activation(sqd_k[:], sbt_sb[:, k * S:(k + 1) * S],
                                     mybir.ActivationFunctionType.Square)
                sqd.append(sqd_k)
            for b in range(32):
                for k in range(2):
                    nc.tensor.matmul(
                        sq_psum[:, b:b + 1],
                        sqd[k][:, b * 128:(b + 1) * 128],
                        ones_kh[:],
                        start=(k == 0), stop=(k == 1),
                    )

            # ---- fp16 running sum of hidden tiles (DVE, DMA-paced) ----
            prev = h_bf[0]
            for n in range(1, 16):
                acc = accp.tile([128, 2048], f16, name="acc")
                nc.vector.tensor_add(acc[:], prev[:], h_bf[n][:])
                prev = acc

            # ---- pooled sums -> pool_chunk (scaled), transposed to fp16 ----
            pool_psum = psum_sm.tile([1, C], f32, name="pool_psum", tag="ps")
            for e in range(8):
                nc.tensor.matmul(pool_psum[:], ones_kh[:],
                                 prev[:, e * C:(e + 1) * C],
                                 start=(e == 0), stop=(e == 7))
            pool_sb = small.tile([1, C], f32)
            nc.vector.tensor_copy(pool_sb[:], pool_psum[:])
            # transpose [1,256] -> [256(2x128), 1] with the 1/N scale fused
            ptp_psum = psum_sm.tile([128, 2], f32, name="ptp_psum", tag="ps")
            for k in range(2):
                nc.tensor.matmul(ptp_psum[:, k:k + 1],
                                 pool_sb[:, k * 128:(k + 1) * 128],
                                 scale_11[:])
            poolT = persist.tile([128, 2], f16)
            nc.vector.tensor_copy(poolT[:], ptp_psum[:])

            # ---- dots on PE: dot[p,b] = sum_c sbt_perm[c, 128b+p]*poolT[c] --
            dot_psum = psum_dot.tile([128, 32], f32)
            for b in range(32):
                for k in range(2):
                    nc.tensor.matmul(
                        dot_psum[:, b:b + 1],
                        sbt_sb[:, k * S + b * 128:k * S + (b + 1) * 128],
                        poolT[:, k:k + 1],
                        start=(k == 0), stop=(k == 1),
                    )
            dmul = small.tile([128, 32], f32)
            nc.vector.tensor_scalar_mul(dmul[:], dot_psum[:], 2.0)
            dloc = persist.tile([128, 32], f32)
            nc.vector.tensor_sub(dloc[:], dmul[:], sq_psum[:])

            # ---- combine partial scores across the 8 column shards ----
            bounce_in = dram.tile([128, 32], f32)
            nc.sync.dma_start(bounce_in[:], dloc[:])
            dred = persist.tile([128, 32], f32)
            bounce_out = dram.tile([128, 32], f32, addr_space="Shared")
            nc.gpsimd.collective_compute(
                "AllReduce",
                mybir.AluOpType.add,
                replica_groups=[list(range(N_CORES))],
                ins=[bounce_in.opt()],
                outs=[bounce_out.opt()],
            )
            nc.sync.dma_start(dred[:], bounce_out[:])

            # ---- global max of scores (PE transpose + DVE reduce) ----
            rowmax = small.tile([128, 1], f32)
            nc.vector.tensor_reduce(rowmax[:], dred[:],
                                    axis=mybir.AxisListType.X,
                                    op=mybir.AluOpType.max)
            rmt_psum = psum_sm.tile([1, 128], f32, name="rmt_psum", tag="ps")
            nc.tensor.transpose(rmt_psum[:], rowmax[:], ident[:])
            rmt_sb = small.tile([1, 128], f32)
            nc.vector.tensor_copy(rmt_sb[:], rmt_psum[:])
            gmax = small.tile([1, 1], f32)
            nc.vector.tensor_reduce(gmax[:], rmt_sb[:],
                                    axis=mybir.AxisListType.X,
                                    op=mybir.AluOpType.max)
            gb_psum = psum_sm.tile([128, 1], f32, name="gb_psum", tag="ps")
            nc.tensor.matmul(gb_psum[:], ones_m[:], gmax[:])
            gmax_bc = small.tile([128, 1], f32)
            nc.vector.tensor_copy(gmax_bc[:], gb_psum[:])

            # ---- one-hot of winner -> seed index ----
            onehot = small.tile([128, 32], f32)
            nc.vector.tensor_scalar(onehot[:], dred[:], gmax_bc[:], None,
                                    op0=mybir.AluOpType.is_equal)
            masked = small.tile([128, 32], f32)
            nc.vector.tensor_mul(masked[:], onehot[:], iota_pb[:])
            idx_rowsum = small.tile([128, 1], f32)
            nc.vector.tensor_reduce(idx_rowsum[:], masked[:],
                                    axis=mybir.AxisListType.X,
                                    op=mybir.AluOpType.add)
            ix_psum = psum_sm.tile([1, 1], f32, name="ix_psum", tag="ps")
            nc.tensor.matmul(ix_psum[:], idx_rowsum[:], ones_k[:])
            idx_sb = small.tile([1, 1], f32)
            nc.vector.tensor_copy(idx_sb[:], ix_psum[:])
            nc.sync.dma_start(sidx.ap(), idx_sb[:])
            idx_i32 = small.tile([1, 1], i32)
            nc.vector.tensor_copy(idx_i32[:], idx_sb[:])

            # ---- usage_new = uf + onehot in natural [32,128] layout ----
            ib_psum = psum_sm.tile([32, 1], f32, name="ib_psum", tag="ps")
            nc.tensor.matmul(ib_psum[:], ones_m[:, :32], idx_sb[:])
            idx_bc = small.tile([32, 1], f32)
            nc.vector.tensor_copy(idx_bc[:], ib_psum[:])
            onehot_nat = small.tile([32, 128], f32)
            nc.vector.tensor_scalar(onehot_nat[:], iota_nat[:], idx_bc[:], None,
                                    op0=mybir.AluOpType.is_equal)
            usage_sb = small.tile([32, 128], f32)
            nc.vector.tensor_add(usage_sb[:], onehot_nat[:], uf_sb[:])
            nc.sync.dma_start(usv[:], usage_sb[:])

            # ---- fetch winning seed row (exact f32) via dynamic-slice DMA --
            row_sb = small.tile([1, C], f32)
            with tc.tile_critical():
                with (
                    nc.gpsimd.register("rowidx") as ridx,
                    nc.semaphore("row_sem") as rsem,
                ):
                    nc.gpsimd.reg_load(ridx, idx_i32[:1, :1])
                    off = nc.gpsimd.snap(ridx)
                    nc.gpsimd.dma_start(
                        row_sb[:], sbk.ap()[bass.ds(off, 1), :]
                    ).then_inc(rsem, 16)
                    nc.gpsimd.wait_ge(rsem, 16)

            # broadcast row to 128 partitions, widen to 8 copies/partition
            rb_psum = psum_sm.tile([128, C], f32, name="rb_psum", tag="ps")
            nc.tensor.matmul(rb_psum[:], ones_m[:], row_sb[:])
            recon_sb = persist.tile([128, 2048], f32)
            nc.vector.tensor_copy(recon_sb[:, :256], rb_psum[:])
            nc.vector.tensor_copy(recon_sb[:, 256:512], recon_sb[:, :256])
            nc.vector.tensor_copy(recon_sb[:, 512:1024], recon_sb[:, :512])
            nc.vector.tensor_copy(recon_sb[:, 1024:2048], recon_sb[:, :1024])

            # ---- write recon chunk: 16 x 1 MiB ----
            for n in range(16):
                nc.sync.dma_start(rv[n], recon_sb[:])

    nc.compile()
    return nc


def _get_program():
    if "nc" not in _CACHE:
        _CACHE["nc"] = _build_program()
    return _CACHE["nc"]


def _seed_perm():
    # column position (128*b + p) of the permuted sbt holds seed s = 128*b + p
    # -> identity; kept for clarity of the layout contract.
    return np.arange(S)


def _shard_inputs(hidden_state, seed_bank, usage_frequency):
    in_maps = []
    for i in range(N_CORES):
        cs = slice(i * C, (i + 1) * C)
        sb_chunk = np.ascontiguousarray(seed_bank[:, cs])
        in_maps.append({
            "h": np.ascontiguousarray(hidden_state[:, cs]),
            "sbk": sb_chunk,
            "sbt": np.ascontiguousarray(sb_chunk.T).astype(np.float16),
            "uf": usage_frequency,
        })
    return in_maps


def kernel(hidden_state, seed_bank, usage_frequency):
    from concourse.bass_utils import run_bass_kernel_spmd

    hidden_state = np.asarray(hidden_state, dtype=np.float32)
    seed_bank = np.asarray(seed_bank, dtype=np.float32)
    usage_frequency = np.asarray(usage_frequency, dtype=np.float32)

    nc = _get_program()
    in_maps = _shard_inputs(hidden_state, seed_bank, usage_frequency)

    res = run_bass_kernel_spmd(nc, in_maps, list(range(N_CORES)))
    results = res.results

    recon = np.concatenate([results[i]["recon"] for i in range(N_CORES)], axis=1)
    usage_new = results[0]["usage"]
    seed_idx = np.int32(np.round(results[0]["sidx"][0, 0]))
    return recon, seed_idx, usage_new


# revision 25
# speedup vs baseline: 1.3299x; 1.3299x over previous
"""Trainium2 Bass kernel for EngramCodebook (vq_codebook).

reference semantics:
    pooled    = hidden_state.mean(axis=0)                 # [2048]
    d[s]      = || seed_bank[s] - pooled ||               # [4096]
    idx       = argmin(d)
    usage_new = usage_frequency + onehot(idx)
    recon     = broadcast(seed_bank[idx], (16384, 2048))

Distribution (8 NeuronCores, column-sharded):
    core i owns columns c_i = [256*i, 256*(i+1)) of hidden_state/seed_bank.
    - local: pool_chunk = mean over rows of hidden[:, c_i]
      (dual-queue loads cast to fp16, DVE running sum, PE ones-matmul)
    - local: score[s] = 2*sb[s,c_i].pool_chunk - ||sb[s,c_i]||^2
      computed entirely from a transposed fp16 seed bank: ACT squares +
      PE M=128 partition-sum matmuls for the norms, PE matmuls against the
      transposed pooled vector for the dots (sign flipped -> arg-MAX).
      fp16 keeps the score error ~0.1 vs a winner gap of ~18.
    - one 16 KB AllReduce combines partial scores across the 8 shards
    - local: max -> index -> dynamic-slice DMA fetches the winning seed row
      (exact f32 from DRAM) -> broadcast to recon[:, c_i]; usage/idx are
      computed identically on every core.

Score-tile layout: element (p, b) of the [128, 32] tiles is seed
    s = 128*b + p  (sbt columns are host-permuted so PE M-blocks line up).
"""

import os
import sys

sys.path.insert(0, "/opt/trn_rl_repo")

import numpy as np

N_CORES = 8
N_ROWS = 16384          # hidden_state rows
D = 2048                # state dim
S = 4096                # num seeds
C = D // N_CORES        # columns per core = 256

_CACHE = {}


def _build_program():
    from concourse import bacc, bass, mybir, tile

    f32 = mybir.dt.float32
    f16 = mybir.dt.float16
    i32 = mybir.dt.int32

    nc = bacc.Bacc("TRN2", target_bir_lowering=False, debug=False,
                   num_devices=N_CORES)

    h = nc.dram_tensor("h", [N_ROWS, C], f32, kind="ExternalInput")
    sbk = nc.dram_tensor("sbk", [S, C], f32, kind="ExternalInput")
    sbt = nc.dram_tensor("sbt", [C, S], f16, kind="ExternalInput")
    uf = nc.dram_tensor("uf", [S], f32, kind="ExternalInput")

    recon = nc.dram_tensor("recon", [N_ROWS, C], f32, kind="ExternalOutput")
    usage = nc.dram_tensor("usage", [S], f32, kind="ExternalOutput")
    sidx = nc.dram_tensor("sidx", [1, 1], f32, kind="ExternalOutput")

    # DRAM views; every DMA has >=1 KiB contiguous runs per partition.
    hv = h.ap().rearrange("(n p e) c -> n p (e c)", n=16, p=128, e=8)
    stv = sbt.ap().rearrange("(k p) s -> k p s", k=2, p=128)
    ufv = uf.ap().rearrange("(q p) -> q p", q=32, p=128)
    usv = usage.ap().rearrange("(q p) -> q p", q=32, p=128)
    rv = recon.ap().rearrange("(n p e) c -> n p (e c)", n=16, p=128, e=8)

    with tile.TileContext(nc) as tc:
        with (
            tc.tile_pool(name="hpool", bufs=4) as hpool,
            tc.tile_pool(name="accp", bufs=2) as accp,
            tc.tile_pool(name="persist", bufs=1) as persist,
            tc.tile_pool(name="small", bufs=1) as small,
            tc.tile_pool(name="psum_dot", bufs=1, space="PSUM") as psum_dot,
            tc.tile_pool(name="psum_sq", bufs=1, space="PSUM") as psum_sq,
            tc.tile_pool(name="psum_sm", bufs=3, space="PSUM") as psum_sm,
            tc.tile_pool(name="dram", bufs=1, space="DRAM") as dram,
        ):
            # ---- kick off the big loads first ----
            # transposed fp16 seed bank (sync queue), persists for the dots
            sbt_sb = persist.tile([128, 2 * S], f16)
            for k in range(2):
                nc.sync.dma_start(sbt_sb[:, k * S:(k + 1) * S], stv[k])

            uf_sb = persist.tile([32, 128], f32)
            nc.sync.dma_start(uf_sb[:], ufv[:])

            # hidden tiles: all on the HWDGE sync queue (f32), cast to fp16
            # on DVE as they land (DVE has ample headroom during the load)
            h_bf = []
            for n in range(16):
                h_f = hpool.tile([128, 2048], f32, name="h_f")
                nc.sync.dma_start(h_f[:], hv[n])
                h_b = hpool.tile([128, 2048], f16, name="h_b")
                nc.vector.tensor_copy(h_b[:], h_f[:])
                h_bf.append(h_b)

            # ---- constants (scheduled into load-phase idle time) ----
            ones_kh = persist.tile([128, 1], f16)
            nc.vector.memset(ones_kh[:], 1.0)
            ones_k = persist.tile([128, 1], f32)
            nc.vector.memset(ones_k[:], 1.0)
            ones_m = persist.tile([1, 128], f32)
            nc.vector.memset(ones_m[:], 1.0)
            scale_11 = persist.tile([1, 1], f32)
            nc.vector.memset(scale_11[:], 1.0 / float(N_ROWS))

            iota_pb_i = small.tile([128, 32], i32)
            nc.gpsimd.iota(iota_pb_i[:], pattern=[[128, 32]], base=0,
                           channel_multiplier=1)
            iota_pb = persist.tile([128, 32], f32)
            nc.vector.tensor_copy(iota_pb[:], iota_pb_i[:])

            iota_nat_i = small.tile([32, 128], i32)
            nc.gpsimd.iota(iota_nat_i[:], pattern=[[1, 128]], base=0,
                           channel_multiplier=128)
            iota_nat = persist.tile([32, 128], f32)
            nc.vector.tensor_copy(iota_nat[:], iota_nat_i[:])

            # identity matrix for PE transposes of [128,1] vectors
            iota_id_i = small.tile([128, 128], i32)
            nc.gpsimd.iota(iota_id_i[:], pattern=[[1, 128]], base=0,
                           channel_multiplier=-1)
            idf = small.tile([128, 128], f32)
            nc.vector.tensor_copy(idf[:], iota_id_i[:])
            ident = persist.tile([128, 128], f32)
            nc.vector.tensor_scalar(ident[:], idf[:], 0.0, None,
                                    op0=mybir.AluOpType.is_equal)

            # warm up the gpsimd dynamic-DMA ucode path during the load so the
            # post-collective row fetch doesn't pay the table-load latency
            warm_sb = small.tile([1, C], f32)
            with tc.tile_critical():
                with (
                    nc.gpsimd.register("warmidx") as widx,
                    nc.semaphore("warm_sem") as wsem,
                ):
                    nc.gpsimd.reg_load(widx, iota_nat_i[:1, :1])
                    woff = nc.gpsimd.snap(widx)
                    nc.gpsimd.dma_start(
                        warm_sb[:], sbk.ap()[bass.ds(woff, 1), :]
                    ).then_inc(wsem, 16)
                    nc.gpsimd.wait_ge(wsem, 16)

            # ---- seed norms from the fp16 transposed bank (load-phase) ----
            # sq[p, b] = sum_c sbt[c, 128b+p]^2 : ACT squares, PE M=128
            # partition-sum matmuls. Runs while the hidden tiles stream in.
            sq_psum = psum_sq.tile([128, 32], f32)
            sqd = []
            for k in range(2):
                sqd_k = persist.tile([128, S], f16, name=f"sqd{k}")
                nc.scalar.activation(sqd_k[:], sbt_sb[:, k * S:(k + 1) * S],
                                     mybir.ActivationFunctionType.Square)
                sqd.append(sqd_k)
            for b in range(32):
                for k in range(2):
                    nc.tensor.matmul(
                        sq_psum[:, b:b + 1],
                        sqd[k][:, b * 128:(b + 1) * 128],
                        ones_kh[:],
                        start=(k == 0), stop=(k == 1),
                    )

            # ---- fp16 running sum of hidden tiles (DVE, DMA-paced) ----
            prev = h_bf[0]
            for n in range(1, 16):
                acc = accp.tile([128, 2048], f16, name="acc")
                nc.vector.tensor_add(acc[:], prev[:], h_bf[n][:])
                prev = acc

            # ---- pooled sums -> pool_chunk (scaled), transposed to fp16 ----
            pool_psum = psum_sm.tile([1, C], f32, name="pool_psum", tag="ps")
            for e in range(8):
                nc.tensor.matmul(pool_psum[:], ones_kh[:],
                                 prev[:, e * C:(e + 1) * C],
                                 start=(e == 0), stop=(e == 7))
            pool_sb = small.tile([1, C], f32)
            nc.vector.tensor_copy(pool_sb[:], pool_psum[:])
            # transpose [1,256] -> [256(2x128), 1] with the 1/N scale fused
            ptp_psum = psum_sm.tile([128, 2], f32, name="ptp_psum", tag="ps")
            for k in range(2):
                nc.tensor.matmul(ptp_psum[:, k:k + 1],
                                 pool_sb[:, k * 128:(k + 1) * 128],
                                 scale_11[:])
            poolT = persist.tile([128, 2], f16)
            nc.vector.tensor_copy(poolT[:], ptp_psum[:])

            # ---- dots on PE: dot[p,b] = sum_c sbt_perm[c, 128b+p]*poolT[c] --
            dot_psum = psum_dot.tile([128, 32], f32)
            for b in range(32):
                for k in range(2):
                    nc.tensor.matmul(
                        dot_psum[:, b:b + 1],
                        sbt_sb[:, k * S + b * 128:k * S + (b + 1) * 128],
                        poolT[:, k:k + 1],
                        start=(k == 0), stop=(k == 1),
                    )
            dmul = small.tile([128, 32], f32)
            nc.vector.tensor_scalar_mul(dmul[:], dot_psum[:], 2.0)
            dloc = persist.tile([128, 32], f32)
            nc.vector.tensor_sub(dloc[:], dmul[:], sq_psum[:])

            # ---- combine partial scores across the 8 column shards ----
            bounce_in = dram.tile([128, 32], f32)
            nc.sync.dma_start(bounce_in[:], dloc[:])
            dred = persist.tile([128, 32], f32)
            bounce_out = dram.tile([128, 32], f32, addr_space="Shared")
            nc.gpsimd.collective_compute(
                "AllReduce",
                mybir.AluOpType.add,
                replica_groups=[list(range(N_CORES))],
                ins=[bounce_in.opt()],
                outs=[bounce_out.opt()],
            )
            nc.sync.dma_start(dred[:], bounce_out[:])

            # ---- global max of scores (PE transpose + DVE reduce) ----
            rowmax = small.tile([128, 1], f32)
            nc.vector.tensor_reduce(rowmax[:], dred[:],
                                    axis=mybir.AxisListType.X,
                                    op=mybir.AluOpType.max)
            rmt_psum = psum_sm.tile([1, 128], f32, name="rmt_psum", tag="ps")
            nc.tensor.transpose(rmt_psum[:], rowmax[:], ident[:])
            rmt_sb = small.tile([1, 128], f32)
            nc.vector.tensor_copy(rmt_sb[:], rmt_psum[:])
            gmax = small.tile([1, 1], f32)
            nc.vector.tensor_reduce(gmax[:], rmt_sb[:],
                                    axis=mybir.AxisListType.X,
                                    op=mybir.AluOpType.max)
            gb_psum = psum_sm.tile([128, 1], f32, name="gb_psum", tag="ps")
            nc.tensor.matmul(gb_psum[:], ones_m[:], gmax[:])
            gmax_bc = small.tile([128, 1], f32)
            nc.vector.tensor_copy(gmax_bc[:], gb_psum[:])

            # ---- one-hot of winner -> seed index ----
            onehot = small.tile([128, 32], f32)
            nc.vector.tensor_scalar(onehot[:], dred[:], gmax_bc[:], None,
                                    op0=mybir.AluOpType.is_equal)
            masked = small.tile([128, 32], f32)
            nc.vector.tensor_mul(masked[:], onehot[:], iota_pb[:])
            idx_rowsum = small.tile([128, 1], f32)
            nc.vector.tensor_reduce(idx_rowsum[:], masked[:],
                                    axis=mybir.AxisListType.X,
                                    op=mybir.AluOpType.add)
            ix_psum = psum_sm.tile([1, 1], f32, name="ix_psum", tag="ps")
            nc.tensor.matmul(ix_psum[:], idx_rowsum[:], ones_k[:])
            idx_sb = small.tile([1, 1], f32)
            nc.vector.tensor_copy(idx_sb[:], ix_psum[:])
            nc.sync.dma_start(sidx.ap(), idx_sb[:])
            idx_i32 = small.tile([1, 1], i32)
            nc.vector.tensor_copy(idx_i32[:], idx_sb[:])

            # ---- usage_new = uf + onehot in natural [32,128] layout ----
            ib_psum = psum_sm.tile([32, 1], f32, name="ib_psum", tag="ps")
            nc.tensor.matmul(ib_psum[:], ones_m[:, :32], idx_sb[:])
            idx_bc = small.tile([32, 1], f32)
            nc.vector.tensor_copy(idx_bc[:], ib_psum[:])
            onehot_nat = small.tile([32, 128], f32)
            nc.vector.tensor_scalar(onehot_nat[:], iota_nat[:], idx_bc[:], None,
                                    op0=mybir.AluOpType.is_equal)
            usage_sb = small.tile([32, 128], f32)
            nc.vector.tensor_add(usage_sb[:], onehot_nat[:], uf_sb[:])
            nc.sync.dma_start(usv[:], usage_sb[:])

            # ---- fetch winning seed row (exact f32) via dynamic-slice DMA --
            row_sb = small.tile([1, C], f32)
            with tc.tile_critical():
                with (
                    nc.gpsimd.register("rowidx") as ridx,
                    nc.semaphore("row_sem") as rsem,
                ):
                    nc.gpsimd.reg_load(ridx, idx_i32[:1, :1])
                    off = nc.gpsimd.snap(ridx)
                    nc.gpsimd.dma_start(
                        row_sb[:], sbk.ap()[bass.ds(off, 1), :]
                    ).then_inc(rsem, 16)
                    nc.gpsimd.wait_ge(rsem, 16)

            # broadcast row to 128 partitions, widen to 8 copies/partition
            rb_psum = psum_sm.tile([128, C], f32, name="rb_psum", tag="ps")
            nc.tensor.matmul(rb_psum[:], ones_m[:], row_sb[:])
            recon_sb = persist.tile([128, 2048], f32)
            nc.vector.tensor_copy(recon_sb[:, :256], rb_psum[:])
            nc.vector.tensor_copy(recon_sb[:, 256:512], recon_sb[:, :256])
            nc.vector.tensor_copy(recon_sb[:, 512:1024], recon_sb[:, :512])
            nc.vector.tensor_copy(recon_sb[:, 1024:2048], recon_sb[:, :1024])

            # ---- write recon chunk: 16 x 1 MiB ----
            for n in range(16):
                nc.sync.dma_start(rv[n], recon_sb[:])

    nc.compile()
    return nc


def _get_program():
    if "nc" not in _CACHE:
        _CACHE["nc"] = _build_program()
    return _CACHE["nc"]


def _seed_perm():
    # column position (128*b + p) of the permuted sbt holds seed s = 128*b + p
    # -> identity; kept for clarity of the layout contract.
    return np.arange(S)


def _shard_inputs(hidden_state, seed_bank, usage_frequency):
    in_maps = []
    for i in range(N_CORES):
        cs = slice(i * C, (i + 1) * C)
        sb_chunk = np.ascontiguousarray(seed_bank[:, cs])
        in_maps.append({
            "h": np.ascontiguousarray(hidden_state[:, cs]),
            "sbk": sb_chunk,
            "sbt": np.ascontiguousarray(sb_chunk.T).astype(np.float16),
            "uf": usage_frequency,
        })
    return in_maps


def kernel(hidden_state, seed_bank, usage_frequency):
    from concourse.bass_utils import run_bass_kernel_spmd

    hidden_state = np.asarray(hidden_state, dtype=np.float32)
    seed_bank = np.asarray(seed_bank, dtype=np.float32)
    usage_frequency = np.asarray(usage_frequency, dtype=np.float32)

    nc = _get_program()
    in_maps = _shard_inputs(hidden_state, seed_bank, usage_frequency)

    res = run_bass_kernel_spmd(nc, in_maps, list(range(N_CORES)))
    results = res.results

    recon = np.concatenate([results[i]["recon"] for i in range(N_CORES)], axis=1)
    usage_new = results[0]["usage"]
    seed_idx = np.int32(np.round(results[0]["sidx"][0, 0]))
    return recon, seed_idx, usage_new
